# revision 8
# baseline (speedup 1.0000x reference)
"""Trainium2 Bass kernel for nn_Decoder (LSTM decoder + 2D-GMM log-prob).

Sharding: data-parallel over K*B=4096 rows, 512 rows/core on 8 cores.
Scan layout: transposed (state dims on partitions, batch on free dim).
Gate pre-activations accumulate differentially in PSUM across timesteps:
  psum_t = psum_{t-1} + Whh@(h_{t-1}-h_{t-2}) + Wd@(d_t - d_{t-1})
so the constant zx-term enters only at t=0 and biases fold into the
activation instruction's per-partition bias operand.
"""
import sys
import numpy as np

sys.path.insert(0, '/opt/trn_rl_repo')
sys.path.insert(0, '/root/.axon_site/_ro/pypackages')

B, K, T, H, GC, PD = 256, 16, 50, 64, 16, 2
XS, ZS = 256, 64
ZX = XS + ZS            # 320
NLOC = 512              # rows per core
NCORE = 8
NCHUNK = 4              # 128-row chunks per core
GRP = 5                 # head-psum timesteps per bank
LOG2PI = float(np.log(2.0 * np.pi))

_CACHE = {}


def _apply_tile_patch():
    """walrus here rejects >1 sem wait on the tile end-drain; spread the
    global-clock waits over NOPs (1 wait each), then drain bare."""
    import concourse.mybir as mybir
    import concourse.tile as tile
    from concourse.vector_clock import ScopedClock
    if getattr(tile.TileContext, '_drain_patched', False):
        return

    def _patched(self, tick_clock, wait_clock):
        nc = self.nc
        carrier = nc.sync.nop()
        wait_clock.add_sem_waits(carrier.ins,
                                 ScopedClock({None: tick_clock.global_clock}))
        si = carrier.ins.sync_info
        waits = list(si.on_wait) if si is not None else []
        ups = list(si.on_update) if si is not None else []
        if len(waits) > 1:
            carrier.ins.sync_info = mybir.SyncInfo(on_wait=waits[:1], on_update=ups)
            for w in waits[1:]:
                n = nc.sync.nop()
                n.ins.sync_info = mybir.SyncInfo(on_wait=[w], on_update=[])
        nc.sync.drain()
        nc.all_engine_barrier()
        popped = nc._tile_sem_poison_stack.pop()
        assert popped is self._sem_poison
        nc.clear_and_free_semaphores(list(self.sems.allocated().values()))
        nc.all_engine_barrier()

    tile.TileContext._drain_and_barrier = _patched

    # This walrus also accepts at most 1 sem wait on ANY instruction: hoist
    # excess waits onto same-engine NoOps committed just before.
    orig_commit = tile.TileContext._commit_instruction

    def _commit_split(self, inst, lazy_reg_writes=True):
        si = getattr(inst, 'sync_info', None)
        if si is not None and si.on_wait and len(si.on_wait) > 1:
            waits = list(si.on_wait)
            for w in waits[:-1]:
                nop = mybir.InstNoOp(
                    name=self.nc.get_next_instruction_name(), ins=[], outs=[])
                nop.engine = inst.engine
                nop.sync_info = mybir.SyncInfo(on_wait=[w], on_update=[])
                orig_commit(self, nop, lazy_reg_writes)
            inst.sync_info = mybir.SyncInfo(on_wait=[waits[-1]],
                                            on_update=list(si.on_update))
        return orig_commit(self, inst, lazy_reg_writes)

    tile.TileContext._commit_instruction = _commit_split
    tile.TileContext._drain_patched = True


def _build_nc():
    import concourse.bass as bass
    import concourse.mybir as mybir
    import concourse.tile as tile
    _apply_tile_patch()
    f32 = mybir.dt.float32
    f32r = mybir.dt.float32r
    bf16 = mybir.dt.bfloat16
    F = mybir.ActivationFunctionType
    A = mybir.AluOpType
    X = mybir.AxisListType.X

    nc = bass.Bass()
    din = lambda n, s: nc.dram_tensor(n, s, f32, kind="ExternalInput")
    zxt = din("zxt", [ZX, NLOC])        # zx transposed, per-core
    wzx = din("wzx", [ZX, 4 * H])       # Wih[:, :ZX].T
    wh0 = din("wh0", [ZX, H])           # Wh0.T
    wc0 = din("wc0", [ZX, H])           # Wc0.T
    wg = din("wg", [H + 2, 4 * H])      # [Whh | Wd].T
    wa = din("wa", [H, 96])             # W_all.T (heads)
    bif = din("bif", [128, 1])          # (bih+bhh)[0:128]
    bgo = din("bgo", [128, 1])          # (bih+bhh)[128:256]
    bh0 = din("bh0", [H, 1])
    bc0 = din("bc0", [H, 1])
    brep5 = din("brep5", [128, GRP * 96])   # head biases tiled x5, bcast 128p
    dd = din("dd", [T, 2, NLOC])        # delta of decode inputs, transposed
    vv = din("vv", [NCHUNK, 128, T * PD])   # GMM target values per chunk
    out = nc.dram_tensor("out", [NCHUNK, 128], f32, kind="ExternalOutput")

    r = lambda ap: ap.bitcast(f32r)
    KCH = [(0, 128), (128, 128), (256, 64)]  # ZX row chunks

    with tile.TileContext(nc) as tc:
        with (
            tc.tile_pool(name="const", bufs=1) as cp,
            tc.tile_pool(name="state", bufs=1) as sp,
            tc.tile_pool(name="haug", bufs=4) as hp,
            tc.tile_pool(name="scr", bufs=2) as scp,
            tc.tile_pool(name="pall", bufs=1) as pp,
            tc.tile_pool(name="gsc", bufs=1) as gp,
            tc.tile_pool(name="psg", bufs=1, space=bass.MemorySpace.PSUM) as psg,
            tc.tile_pool(name="psh", bufs=1, space=bass.MemorySpace.PSUM) as psh,
        ):
            # ---- load constants ----
            zxc = []
            for (o, k) in KCH:
                t_ = cp.tile([k, NLOC], f32r, tag=f"zx{o}")
                nc.gpsimd.dma_start(t_[:], zxt[o:o + k, :])
                zxc.append(t_)
            wzxc, wh0c, wc0c = [], [], []
            for (o, k) in KCH:
                t_ = cp.tile([k, 4 * H], f32r, tag=f"wzx{o}")
                nc.gpsimd.dma_start(t_[:], wzx[o:o + k, :])
                wzxc.append(t_)
                t_ = cp.tile([k, H], f32r, tag=f"wh0{o}")
                nc.gpsimd.dma_start(t_[:], wh0[o:o + k, :])
                wh0c.append(t_)
                t_ = cp.tile([k, H], f32r, tag=f"wc0{o}")
                nc.gpsimd.dma_start(t_[:], wc0[o:o + k, :])
                wc0c.append(t_)
            wgR = cp.tile([H + 2, 4 * H], f32r)
            nc.gpsimd.dma_start(wgR[:], wg[:])
            wat = cp.tile([H, 96], f32)
            nc.sync.dma_start(wat[:], wa[:])
            wab = cp.tile([H, 96], bf16)
            nc.vector.tensor_copy(wab[:], wat[:])
            bift = cp.tile([128, 1], f32)
            nc.sync.dma_start(bift[:], bif[:])
            bgot = cp.tile([128, 1], f32)
            nc.sync.dma_start(bgot[:], bgo[:])
            bh0t = cp.tile([H, 1], f32)
            nc.sync.dma_start(bh0t[:], bh0[:])
            bc0t = cp.tile([H, 1], f32)
            nc.sync.dma_start(bc0t[:], bc0[:])
            brt = cp.tile([128, GRP * 96], f32)
            nc.sync.dma_start(brt[:], brep5[:])
            vt = []
            for q in range(NCHUNK):
                t_ = cp.tile([128, T * PD], f32, tag=f"v{q}")
                nc.sync.dma_start(t_[:], vv[q])
                vt.append(t_)

            # ---- persistent state ----
            ps_if = psg.tile([128, NLOC], f32, tag="ps_if")
            ps_go = psg.tile([128, NLOC], f32, tag="ps_go")
            c_t = sp.tile([H, NLOC], f32, tag="c")
            h_st = [sp.tile([H, NLOC], f32, tag=f"h{i}", name=f"h{i}") for i in range(2)]
            pall = [pp.tile([128, T * 96], f32, tag=f"pa{q}", name=f"pa{q}") for q in range(NCHUNK)]

            # ---- prologue: G0 into gate psum banks; h0/c0 ----
            for half, ps in ((0, ps_if), (1, ps_go)):
                for i, (o, k) in enumerate(KCH):
                    nc.tensor.matmul(
                        ps[:], wzxc[i][:, 128 * half:128 * half + 128],
                        zxc[i][:], start=(i == 0), stop=(i == 2))
            ps0 = psh.tile([H, NLOC], f32, tag="ph0")
            for i, (o, k) in enumerate(KCH):
                nc.tensor.matmul(ps0[:], wh0c[i][:], zxc[i][:],
                                 start=(i == 0), stop=(i == 2))
            ha = hp.tile([H + 2, NLOC], f32r)
            nc.gpsimd.dma_start(ha[H:H + 2, :], dd[0])
            nc.scalar.activation(ha[0:H, :], ps0[:], F.Identity, bias=bh0t[:])
            nc.scalar.activation(h_st[1][:], ps0[:], F.Identity, bias=bh0t[:])
            ps0b = psh.tile([H, NLOC], f32, tag="ph1")
            for i, (o, k) in enumerate(KCH):
                nc.tensor.matmul(ps0b[:], wc0c[i][:], zxc[i][:],
                                 start=(i == 0), stop=(i == 2))
            nc.scalar.activation(c_t[:], ps0b[:], F.Identity, bias=bc0t[:])

            # ---- scan over T ----
            ph = None
            for t in range(T):
                nc.tensor.matmul(ps_if[:], wgR[:, 0:128], ha[:],
                                 start=False, stop=True,
                                 skip_group_check=True)
                nc.tensor.matmul(ps_go[:], wgR[:, 128:256], ha[:],
                                 start=False, stop=True,
                                 skip_group_check=True)
                # sigma(i),sigma(f) -> PSUM so the f-half (partitions 64:128)
                # can meet partition-0 SBUF tiles (PSUM APs are exempt from
                # the same-start-partition rule)
                sig_if = psh.tile([128, NLOC], f32, tag="aif", name=f"aif{t}")
                nc.scalar.activation(sig_if[:], ps_if[:], F.Sigmoid,
                                     bias=bift[:])
                tg = scp.tile([H, NLOC], f32, tag="tg")
                nc.scalar.activation(tg[:], ps_go[0:H, :], F.Tanh,
                                     bias=bgot[0:H, :])
                sig_o = psh.tile([H, NLOC], f32, tag="aso", name=f"aso{t}")
                nc.scalar.activation(sig_o[:], ps_go[H:128, :], F.Sigmoid,
                                     bias=bgot[H:128, :])
                m1 = scp.tile([H, NLOC], f32, tag="m1")
                nc.vector.tensor_tensor(m1[:], sig_if[0:H, :], tg[:], A.mult)
                nc.vector.tensor_tensor(c_t[:], sig_if[H:128, :], c_t[:],
                                        A.mult)
                nc.vector.tensor_tensor(c_t[:], c_t[:], m1[:], A.add)
                tcn = scp.tile([H, NLOC], f32, tag="tc")
                nc.scalar.activation(tcn[:], c_t[:], F.Tanh)
                hnew = h_st[t % 2]
                hprev = h_st[(t + 1) % 2]
                nc.vector.tensor_tensor(hnew[:], sig_o[:], tcn[:], A.mult)
                # heads (bf16) into grouped psum banks
                if t % GRP == 0:
                    ph = [psh.tile([128, GRP * 96], f32, tag=f"ph{q}", name=f"ph{q}_{t}")
                          for q in range(NCHUNK)]
                hbf = scp.tile([H, NLOC], bf16, tag="hbf")
                nc.gpsimd.tensor_copy(hbf[:], hnew[:])
                g, s = t // GRP, t % GRP
                for q in range(NCHUNK):
                    nc.tensor.matmul(ph[q][:, 96 * s:96 * (s + 1)],
                                     hbf[:, 128 * q:128 * (q + 1)], wab[:],
                                     start=True, stop=True)
                if s == GRP - 1:
                    for q in range(NCHUNK):
                        nc.vector.tensor_tensor(
                            pall[q][:, 480 * g:480 * (g + 1)], ph[q][:],
                            brt[:], A.add)
                # delta-h into next step's augmented rhs
                if t < T - 1:
                    ha = hp.tile([H + 2, NLOC], f32r)
                    nc.gpsimd.dma_start(ha[H:H + 2, :], dd[t + 1])
                    nc.gpsimd.tensor_tensor(ha[0:H, :], hnew[:], hprev[:],
                                            A.subtract)

            # ---- GMM phase, per 128-row chunk ----
            for q in range(NCHUNK):
                P3 = pall[q][:].rearrange("p (t c) -> p t c", c=96)
                PI = P3[:, :, 0:16]
                MU = P3[:, :, 16:48].rearrange("p t (g d) -> p t g d", d=2)
                LS = P3[:, :, 48:80]
                CO = P3[:, :, 80:96]
                vb = vt[q][:].rearrange("p (t d) -> p t d", d=2)
                vb4 = vb.unsqueeze(2).broadcast_to([128, T, GC, PD])
                g32 = lambda nm: gp.tile([128, T * 32], f32, tag="g32",
                                         bufs=5, name=nm)
                g16 = lambda nm: gp.tile([128, T * GC], f32, tag="g16",
                                         bufs=8, name=nm)
                gs = lambda nm: gp.tile([128, T], f32, tag="gs", bufs=8,
                                        name=nm)
                r4 = lambda ap: ap.rearrange("p (t g d) -> p t g d", g=GC, d=2)
                r3 = lambda ap: ap.rearrange("p (t g) -> p t g", g=GC)

                lsc = g32("lsc")
                nc.vector.tensor_scalar(lsc[:].rearrange("p (t c) -> p t c", c=32),
                                        LS, 10.0, -10.0, A.min, A.max)
                av = g32("av")
                nc.scalar.activation(av[:], lsc[:], F.Exp, scale=-1.0)
                dx = g32("dx")
                nc.vector.tensor_tensor(r4(dx[:]), vb4, MU, A.subtract)
                zs = g32("zs")
                nc.vector.tensor_tensor(zs[:], dx[:], av[:], A.mult)
                zs4 = r4(zs[:])
                # g16 chain; 8-slot rotation, lifetimes all < 8 allocs
                ct = g16("ct")
                nc.scalar.activation(r3(ct[:]), CO, F.Tanh)
                rr2 = g16("rr2")
                nc.gpsimd.tensor_tensor(rr2[:], ct[:], ct[:], A.mult)
                omr = g16("omr")
                nc.vector.tensor_scalar(omr[:], rr2[:], -1.0, 1.0, A.mult, A.add)
                rec = g16("rec")
                nc.vector.reciprocal(rec[:], omr[:])
                lno = g16("lno")
                nc.scalar.activation(lno[:], omr[:], F.Ln)
                zz = g16("zz")
                nc.gpsimd.tensor_tensor(r3(zz[:]), zs4[:, :, :, 0],
                                        zs4[:, :, :, 1], A.mult)
                rz = g16("rz")
                nc.gpsimd.tensor_tensor(rz[:], ct[:], zz[:], A.mult)
                sq = g32("sq")
                nc.vector.tensor_tensor(sq[:], zs[:], zs[:], A.mult)
                s2 = g16("s2")
                nc.vector.tensor_reduce(r3(s2[:]).unsqueeze(3), r4(sq[:]),
                                        X, A.add)
                quad = g16("quad")
                nc.vector.scalar_tensor_tensor(quad[:], rz[:], -2.0, s2[:],
                                               A.mult, A.add)
                qd = g16("qd")
                nc.vector.tensor_tensor(qd[:], quad[:], rec[:], A.mult)
                e1 = g16("e1")
                nc.vector.tensor_tensor(e1[:], qd[:], lno[:], A.add)
                lss = g16("lss")
                lsc4 = r4(lsc[:])
                nc.vector.tensor_reduce(r3(lss[:]).unsqueeze(3), lsc4, X, A.add)
                e2 = g16("e2")
                nc.vector.tensor_scalar(e2[:], e1[:], -0.5, -LOG2PI,
                                        A.mult, A.add)
                arg = g16("arg")
                nc.vector.tensor_tensor(arg[:], e2[:], lss[:], A.subtract)
                nc.gpsimd.tensor_tensor(r3(arg[:]), r3(arg[:]), PI, A.add)
                ex1 = g16("ex1")
                nc.scalar.activation(ex1[:], arg[:], F.Exp)
                ex0 = g16("ex0")
                nc.scalar.activation(r3(ex0[:]), PI, F.Exp)
                s1 = gs("s1")
                nc.vector.tensor_reduce(s1[:].unsqueeze(2), r3(ex1[:]), X, A.add)
                s0 = gs("s0")
                nc.vector.tensor_reduce(s0[:].unsqueeze(2), r3(ex0[:]), X, A.add)
                l1 = gs("l1")
                nc.scalar.activation(l1[:], s1[:], F.Ln)
                l0 = gs("l0")
                nc.scalar.activation(l0[:], s0[:], F.Ln)
                lp = gs("lp")
                nc.vector.tensor_tensor(lp[:], l1[:], l0[:], A.subtract)
                nc.vector.tensor_scalar_min(lp[:], lp[:], 50.0)
                res = gs("res")
                nc.vector.tensor_reduce(res[:, 0:1], lp[:], X, A.add)
                nc.sync.dma_start(out[q].unsqueeze(1), res[:, 0:1])
    return nc


def _prep_inputs(x, z_stacked, input_seqs, pred_seqs,
                 Wh0, bh0, Wc0, bc0, Wih, Whh, bih, bhh,
                 Wpi, bpi, Wmu, bmu, Wls, bls, Wcorr, bcorr, n_z_samples):
    f = np.float32
    tgt_future = np.ascontiguousarray(pred_seqs[:, :, 1, 1, 2:4], f)  # [B,T,2]
    tgt_present = np.ascontiguousarray(input_seqs[:, -1, 1, 1, 2:4], f)  # [B,2]
    z = z_stacked.reshape(-1, ZS).astype(f)
    zx = np.concatenate([z, np.tile(x.astype(f), (K, 1))], axis=1)  # [4096,ZX]
    decode = np.concatenate([tgt_present[:, None, :], tgt_future[:, :T - 1, :]],
                            axis=1)                                  # [B,T,2]
    dcore = np.tile(decode, (2, 1, 1))                               # [512,T,2]
    dT = np.ascontiguousarray(dcore.transpose(1, 2, 0))              # [T,2,512]
    ddel = dT.copy()
    ddel[1:] -= dT[:-1]
    vcore = np.tile(tgt_future, (2, 1, 1)).reshape(NCHUNK, 128, T * PD)
    b_all = np.concatenate([bpi, bmu, bls, bcorr]).astype(f)         # [96]
    w_all = np.concatenate([Wpi, Wmu, Wls, Wcorr], axis=0).astype(f)  # [96,H]
    bb = (bih + bhh).astype(f)
    common = {
        "wzx": np.ascontiguousarray(Wih[:, :ZX].T, f),
        "wh0": np.ascontiguousarray(Wh0.T, f),
        "wc0": np.ascontiguousarray(Wc0.T, f),
        "wg": np.ascontiguousarray(
            np.concatenate([Whh.T, Wih[:, ZX:ZX + 2].T], axis=0), f),
        "wa": np.ascontiguousarray(w_all.T, f),
        "bif": np.ascontiguousarray(bb[:128, None], f),
        "bgo": np.ascontiguousarray(bb[128:, None], f),
        "bh0": np.ascontiguousarray(bh0[:, None], f),
        "bc0": np.ascontiguousarray(bc0[:, None], f),
        "brep5": np.ascontiguousarray(
            np.broadcast_to(np.tile(b_all, GRP)[None, :], (128, GRP * 96)), f),
        "dd": np.ascontiguousarray(ddel, f),
        "vv": np.ascontiguousarray(vcore, f),
    }
    maps = []
    for c in range(NCORE):
        m = dict(common)
        m["zxt"] = np.ascontiguousarray(
            zx[c * NLOC:(c + 1) * NLOC].T, f)      # [ZX,512]
        maps.append(m)
    return maps


def kernel(**inputs):
    from concourse.bass_utils import run_bass_kernel_spmd
    if "nc" not in _CACHE:
        _CACHE["nc"] = _build_nc()
    nc = _CACHE["nc"]
    maps = _prep_inputs(**{k: np.asarray(v) for k, v in inputs.items()})
    res = run_bass_kernel_spmd(nc, maps, list(range(NCORE)))
    full = np.concatenate([res.results[c]["out"].reshape(NLOC)
                           for c in range(NCORE)])
    return full.reshape(K, B).astype(np.float32)
